# revision 18
# baseline (speedup 1.0000x reference)
"""Trainium2 Bass kernel for nn_MaxExtractor (masked pairwise-IoU max + union max).

Contract: kernel(**inputs) takes FULL unsharded inputs, returns the FULL [2]
output. Internally shards the batch dim (8 images) across 8 NeuronCores, one
image per core; each core computes [max_prob_t, iou_max_of_its_image]; the
host gathers and averages the per-image iou scalars.

Per-core algorithm (N=4096 preds, M=2048 gts):
  Only preds with class==0 (~50/4096) matter, so the core first compacts
  person preds on-device: mask -> free-dim cumsum (tensor_tensor_scan) ->
  cross-partition exclusive prefix (triangular matmul) -> ranks -> one-hot
  -> 32 accumulating PE matmuls gather person boxes into [128, 4].
  Slot layout: partition p holds person (p % K) and gt-half (p // K), K=64,
  so the pairwise phase is [128, 1024] instead of [128, 2048].
  Pairwise: iw = min(px2,gx2)-max(px1,gx1) etc. via fused DVE ops; instead of
  iou = inter/uni per pair, rank by r = inter/(area_p+area_g+eps) which is a
  monotone transform of iou (iou = r/(1-r)); one reciprocal_approx_fast gives
  the 1/(ap+ag) tensor. Final: partition all-reduce max, iou = r*/(1-r*).
"""

import sys

sys.path.insert(0, "/opt/trn_rl_repo")

import contextlib

import numpy as np

import concourse.bacc as bacc
import concourse.mybir as mybir
from concourse import bass_isa
from concourse.tile import TileContext

F32 = mybir.dt.float32
I32 = mybir.dt.int32
Alu = mybir.AluOpType
Act = mybir.ActivationFunctionType

N = 4096  # preds per image
M = 2048  # gts per image
B = 8  # images == cores
U = 4096  # union entries
BIG = 1.0e30
EPS = 1.0e-9
FDB = 1024  # gt-block free size for the pairwise phase


def build_kernel(K: int, debug_taps: bool = False):
    """Build the per-core Bass module. K = person-slot count (64 or 128).

    K=64: partition p = (gt-half p//64, person p%64); one gt block of 1024.
    K=128: partition p = person p; two sequential gt blocks of 1024.
    """
    assert K in (64, 128)
    nhalf = 128 // K  # gt-halves packed along partitions
    nblk = M // (nhalf * FDB)  # sequential gt blocks
    NCH = 32  # pred chunks of 128 (contract dim)

    nc = bacc.Bacc("TRN2", target_bir_lowering=False, debug=False)

    pred_boxes = nc.dram_tensor("pred_boxes", [N, 4], F32, kind="ExternalInput")
    pred_classes = nc.dram_tensor("pred_classes", [N], I32, kind="ExternalInput")
    gt_boxes = nc.dram_tensor("gt_boxes", [M, 4], F32, kind="ExternalInput")
    union_scores = nc.dram_tensor("union_scores", [U], F32, kind="ExternalInput")
    union_classes = nc.dram_tensor("union_classes", [U], I32, kind="ExternalInput")
    out = nc.dram_tensor("out", [2], F32, kind="ExternalOutput")
    taps = {}
    if debug_taps:
        for tname, shape in [
            ("d_rmaxg", [128, 1]), ("d_onem", [1, 1]),
            ("d_q", [128, 32]), ("d_pc", [128, 4]), ("d_apeps", [128, 1]),
            ("d_m2x", [128, FDB]), ("d_zx", [128, FDB]), ("d_iwr", [128, FDB]),
            ("d_ihr", [128, FDB]), ("d_srecip", [128, FDB]), ("d_rbuf", [128, FDB]),
            ("d_rmax", [128, 1]),
        ]:
            taps[tname] = nc.dram_tensor(tname, shape, F32, kind="ExternalOutput")

    # ---- constants baked into the NEFF ----
    # one-hot targets: slot p matches rank (p % K) + 1; laid out on free dim
    iota_np = np.tile(np.arange(1, K + 1, dtype=np.float32), 128 // K)  # [128]
    iota_const = nc.inline_tensor(np.broadcast_to(iota_np, (128, 128)).copy(), "iota_ranks")
    # strict lower-triangular for cross-partition exclusive prefix sum
    tri_np = (np.arange(128)[:, None] < np.arange(128)[None, :]).astype(np.float32)
    tri_const = nc.inline_tensor(tri_np, "tri128")
    # gt-half selector for broadcasts: col p -> onehot(p // K)
    sel_np = np.zeros((nhalf, 128), dtype=np.float32)
    for p in range(128):
        sel_np[(p // K) % nhalf, p] = 1.0
    sel_const = nc.inline_tensor(sel_np, "sel_half")

    with TileContext(nc) as tc:
        ctx = contextlib.ExitStack()
        with ctx:
            const_pool = ctx.enter_context(tc.tile_pool(name="const", bufs=1))
            sb = ctx.enter_context(tc.tile_pool(name="sbuf", bufs=1))
            ohp = ctx.enter_context(tc.tile_pool(name="ohp", bufs=4))
            small = ctx.enter_context(tc.tile_pool(name="small", bufs=1))
            ps = ctx.enter_context(tc.tile_pool(name="ps", bufs=4, space="PSUM"))

            # ---------------- loads ----------------
            cls_sb = sb.tile([128, 32], I32, tag="cls")
            nc.sync.dma_start(
                out=cls_sb[:], in_=pred_classes.ap().rearrange("(p f) -> p f", p=128)
            )
            iota_sb = const_pool.tile([128, 128], F32, tag="iota")
            nc.sync.dma_start(out=iota_sb[:], in_=iota_const.ap())
            tri_sb = const_pool.tile([128, 128], F32, tag="tri")
            nc.sync.dma_start(out=tri_sb[:], in_=tri_const.ap())

            pbox_sb = sb.tile([128, 128], F32, tag="pbox")
            nc.sync.dma_start(
                out=pbox_sb[:],
                in_=pred_boxes.ap().flatten().rearrange("(p f) -> p f", p=128),
            )
            # gt boxes, two layouts: [nhalf, 4M/nhalf] rows for PE broadcast
            # and [128, 4M/128] compact for area computation
            grow = sb.tile([nhalf, 4 * M // nhalf], F32, tag="grow")
            nc.sync.dma_start(
                out=grow[:],
                in_=gt_boxes.ap().flatten().rearrange("(q x) -> q x", q=nhalf),
            )
            uscore = sb.tile([128, U // 128], F32, tag="uscore")
            nc.sync.dma_start(
                out=uscore[:], in_=union_scores.ap().rearrange("(p f) -> p f", p=128)
            )
            ucls = sb.tile([128, U // 128], I32, tag="ucls")
            nc.sync.dma_start(
                out=ucls[:], in_=union_classes.ap().rearrange("(p f) -> p f", p=128)
            )
            sel_sb = const_pool.tile([nhalf, 128], F32, tag="sel")
            nc.sync.dma_start(out=sel_sb[:], in_=sel_const.ap())

            # ---------------- person mask + ranks ----------------
            clsf = small.tile([128, 32], F32, tag="clsf")
            nc.vector.tensor_copy(clsf[:], cls_sb[:])  # i32 -> f32 cast
            m = small.tile([128, 32], F32, tag="m")
            nc.vector.tensor_scalar(m[:], clsf[:], 0.0, None, Alu.is_equal)
            # inclusive cumsum along free dim: state = (m + state) max m (state>=0)
            s = small.tile([128, 32], F32, tag="s")
            nc.vector.tensor_tensor_scan(s[:], m[:], m[:], 0.0, Alu.add, Alu.max)
            # cross-partition exclusive prefix of per-partition totals
            pref_ps = ps.tile([128, FDB], F32, tag="ps")
            nc.tensor.matmul(
                pref_ps[:, 0:1], tri_sb[:], s[:, 31:32], start=True, stop=True
            )
            # global inclusive rank, then zero for non-persons
            sg = small.tile([128, 32], F32, tag="sg")
            nc.vector.tensor_scalar(sg[:], s[:], pref_ps[:, 0:1], None, Alu.add)
            q = small.tile([128, 32], F32, tag="q")
            nc.vector.tensor_mul(q[:], sg[:], m[:])
            if debug_taps:
                nc.sync.dma_start(out=taps["d_q"].ap(), in_=q[:])

            # ---------------- compaction: one-hot + matmul gather ----------------
            pc_ps = ps.tile([128, FDB], F32, tag="ps")
            for f in range(NCH):
                oh = ohp.tile([128, 128], F32, tag="oh")
                nc.vector.tensor_scalar(
                    oh[:], iota_sb[:], q[:, f : f + 1], None, Alu.is_equal
                )
                nc.tensor.matmul(
                    pc_ps[:, 0:4],
                    oh[:],
                    pbox_sb[:, 4 * f : 4 * f + 4],
                    start=(f == 0),
                    stop=(f == NCH - 1),
                )
            pc = small.tile([128, 4], F32, tag="pcs")
            nc.vector.tensor_copy(pc[:], pc_ps[:, 0:4])
            px1, py1, px2, py2 = (pc[:, i : i + 1] for i in range(4))
            # person area + eps (per-partition scalar)
            wp = small.tile([128, 1], F32, tag="wp")
            nc.vector.tensor_sub(wp[:], px2, px1)
            hp = small.tile([128, 1], F32, tag="hp")
            nc.vector.tensor_sub(hp[:], py2, py1)
            ap_eps = small.tile([128, 1], F32, tag="ap_eps")
            nc.vector.scalar_tensor_tensor(
                ap_eps[:], wp[:], EPS, hp[:], Alu.bypass, Alu.mult
            )
            nc.vector.tensor_scalar(ap_eps[:], ap_eps[:], EPS, None, Alu.add)
            if debug_taps:
                nc.sync.dma_start(out=taps["d_pc"].ap(), in_=pc[:])
                nc.sync.dma_start(out=taps["d_apeps"].ap(), in_=ap_eps[:])

            # ---------------- gt areas (row layout, strided views) ----------------
            GWALL = 4 * M // nhalf  # grow free width
            MR = M // nhalf  # gts per row
            wg_r = small.tile([nhalf, MR], F32, tag="wg_r")
            nc.gpsimd.tensor_sub(wg_r[:], grow[:, 2:GWALL:4], grow[:, 0:GWALL:4])
            hg_r = small.tile([nhalf, MR], F32, tag="hg_r")
            nc.gpsimd.tensor_sub(hg_r[:], grow[:, 3:GWALL:4], grow[:, 1:GWALL:4])
            ag_row = sb.tile([nhalf, MR], F32, tag="ag_row")
            nc.gpsimd.tensor_mul(ag_row[:], wg_r[:], hg_r[:])

            # ---------------- per gt-block: broadcast + pairwise ----------------
            GWB = 4 * FDB  # grow elems per block per half
            rmax_prev = None
            for blk in range(nblk):
                g0 = blk * GWB
                ag_ps = ps.tile([128, FDB], F32, tag="ps")
                for h in range(0, FDB, 512):
                    nc.tensor.matmul(
                        ag_ps[:, h : h + 512],
                        sel_sb[:],
                        ag_row[:, blk * FDB + h : blk * FDB + h + 512],
                        start=True,
                        stop=True,
                    )
                S_sb = sb.tile([128, FDB], F32, tag="S")
                nc.scalar.activation(
                    S_sb[:], ag_ps[:], Act.Identity, bias=ap_eps[:], scale=1.0
                )
                srecip = sb.tile([128, FDB], F32, tag="srecip")
                nc.vector.reciprocal_approx_fast(srecip[:], S_sb[:])

                gx1_ps = ps.tile([128, FDB], F32, tag="ps")
                for h in range(0, FDB, 512):
                    nc.tensor.matmul(
                        gx1_ps[:, h : h + 512],
                        sel_sb[:],
                        grow[:, g0 + 4 * h + 0 : g0 + 4 * h + 2048 : 4],
                        start=True, stop=True,
                    )
                gx2_ps = ps.tile([128, FDB], F32, tag="ps")
                for h in range(0, FDB, 512):
                    nc.tensor.matmul(
                        gx2_ps[:, h : h + 512],
                        sel_sb[:],
                        grow[:, g0 + 4 * h + 2 : g0 + 4 * h + 2048 : 4],
                        start=True, stop=True,
                    )
                gy1_ps = ps.tile([128, FDB], F32, tag="ps")
                for h in range(0, FDB, 512):
                    nc.tensor.matmul(
                        gy1_ps[:, h : h + 512],
                        sel_sb[:],
                        grow[:, g0 + 4 * h + 1 : g0 + 4 * h + 2048 : 4],
                        start=True, stop=True,
                    )
                gy2_ps = ps.tile([128, FDB], F32, tag="ps")
                for h in range(0, FDB, 512):
                    nc.tensor.matmul(
                        gy2_ps[:, h : h + 512],
                        sel_sb[:],
                        grow[:, g0 + 4 * h + 3 : g0 + 4 * h + 2048 : 4],
                        start=True, stop=True,
                    )

                m2x = sb.tile([128, FDB], F32, tag="m2x")
                nc.vector.tensor_scalar(m2x[:], gx1_ps[:], px1, None, Alu.max)
                zx = sb.tile([128, FDB], F32, tag="zx")
                nc.vector.scalar_tensor_tensor(
                    zx[:], gx2_ps[:], px2, m2x[:], Alu.min, Alu.subtract
                )
                m2y = sb.tile([128, FDB], F32, tag="m2y")
                nc.vector.tensor_scalar(m2y[:], gy1_ps[:], py1, None, Alu.max)
                zy = sb.tile([128, FDB], F32, tag="zy")
                nc.vector.scalar_tensor_tensor(
                    zy[:], gy2_ps[:], py2, m2y[:], Alu.min, Alu.subtract
                )
                iwr = sb.tile([128, FDB], F32, tag="iwr")
                nc.scalar.activation(iwr[:], zx[:], Act.Relu)
                ihr = sb.tile([128, FDB], F32, tag="ihr")
                nc.gpsimd.tensor_relu(ihr[:], zy[:])
                inter = sb.tile([128, FDB], F32, tag="inter")
                nc.vector.tensor_mul(inter[:], iwr[:], ihr[:])
                r_buf = sb.tile([128, FDB], F32, tag="r_buf")
                nc.vector.tensor_mul(r_buf[:], inter[:], srecip[:])
                rmax = small.tile([128, 1], F32, tag=f"rmax{blk}")
                nc.vector.tensor_reduce(rmax[:], r_buf[:], mybir.AxisListType.X, Alu.max)
                if rmax_prev is not None:
                    rmax2 = small.tile([128, 1], F32, tag=f"rmaxc{blk}")
                    nc.vector.tensor_max(rmax2[:], rmax[:], rmax_prev[:])
                    rmax = rmax2
                if debug_taps and blk == 0:
                    nc.sync.dma_start(out=taps["d_m2x"].ap(), in_=m2x[:])
                    nc.sync.dma_start(out=taps["d_zx"].ap(), in_=zx[:])
                    nc.sync.dma_start(out=taps["d_iwr"].ap(), in_=iwr[:])
                    nc.sync.dma_start(out=taps["d_ihr"].ap(), in_=ihr[:])
                    nc.sync.dma_start(out=taps["d_srecip"].ap(), in_=srecip[:])
                    nc.sync.dma_start(out=taps["d_rbuf"].ap(), in_=r_buf[:])
                    nc.sync.dma_start(out=taps["d_rmax"].ap(), in_=rmax[:])
                rmax_prev = rmax

            # ---------------- union max ----------------
            mu = small.tile([128, U // 128], I32, tag="mu")
            nc.vector.tensor_scalar(mu[:], ucls[:], 0, None, Alu.is_equal)
            um = small.tile([128, U // 128], F32, tag="um")
            nc.vector.memset(um[:], -BIG)
            nc.vector.copy_predicated(um[:], mu[:], uscore[:])
            umax = small.tile([128, 1], F32, tag="umax")
            nc.vector.tensor_reduce(umax[:], um[:], mybir.AxisListType.X, Alu.max)

            # ---------------- final reductions ----------------
            rmax_g = small.tile([128, 1], F32, tag="rmax_g")
            nc.gpsimd.partition_all_reduce(
                rmax_g[:], rmax_prev[:], 128, bass_isa.ReduceOp.max
            )
            umax_g = small.tile([128, 1], F32, tag="umax_g")
            nc.gpsimd.partition_all_reduce(
                umax_g[:], umax[:], 128, bass_isa.ReduceOp.max
            )

            # iou = r / (1 - r)
            one_m = small.tile([1, 1], F32, tag="one_m")
            nc.vector.tensor_scalar(
                one_m[:], rmax_g[0:1, :], 1.0, -1.0, Alu.subtract, Alu.mult
            )
            if debug_taps:
                nc.sync.dma_start(out=taps["d_rmaxg"].ap(), in_=rmax_g[:])
                nc.sync.dma_start(out=taps["d_onem"].ap(), in_=one_m[:])
            rec = small.tile([1, 1], F32, tag="rec")
            nc.vector.reciprocal(rec[:], one_m[:])
            res = small.tile([1, 2], F32, tag="res")
            nc.vector.tensor_copy(res[:, 0:1], umax_g[0:1, :])
            nc.vector.tensor_mul(res[:, 1:2], rmax_g[0:1, :], rec[:])

            nc.sync.dma_start(out=out.ap(), in_=res[:])

    nc.compile()
    return nc


_KERNEL_CACHE = {}

# test/dev hooks: set TRACE=True before calling kernel() to capture an NTFF
# profile; the raw BassKernelResults lands in LAST_RESULTS.
TRACE = False
LAST_RESULTS = None


def _get_kernel(K: int):
    if K not in _KERNEL_CACHE:
        _KERNEL_CACHE[K] = build_kernel(K)
    return _KERNEL_CACHE[K]


def kernel(pred_boxes, pred_scores, pred_classes, gt_boxes, union_scores, union_classes):
    from concourse.bass_utils import run_bass_kernel_spmd

    pred_boxes = np.ascontiguousarray(np.asarray(pred_boxes, dtype=np.float32))
    pred_classes = np.ascontiguousarray(np.asarray(pred_classes, dtype=np.int32))
    gt_boxes = np.ascontiguousarray(np.asarray(gt_boxes, dtype=np.float32))
    union_scores = np.ascontiguousarray(np.asarray(union_scores, dtype=np.float32))
    union_classes = np.ascontiguousarray(np.asarray(union_classes, dtype=np.int32))

    max_persons = int((pred_classes == 0).sum(axis=1).max())
    K = 64 if max_persons <= 64 else 128
    nc = _get_kernel(K)

    in_maps = [
        {
            "pred_boxes": pred_boxes[b],
            "pred_classes": pred_classes[b],
            "gt_boxes": gt_boxes[b],
            "union_scores": union_scores,
            "union_classes": union_classes,
        }
        for b in range(B)
    ]
    res = run_bass_kernel_spmd(nc, in_maps, list(range(B)), trace=TRACE)
    global LAST_RESULTS
    LAST_RESULTS = res
    outs = np.stack([res.results[b]["out"] for b in range(B)])  # [B, 2]
    max_prob = outs[0, 0]
    max_iou = outs[:, 1].mean(dtype=np.float32)
    return np.array([max_prob, max_iou], dtype=np.float32)


# revision 23
# speedup vs baseline: 1.3539x; 1.3539x over previous
"""Trainium2 Bass kernel for nn_MaxExtractor (masked pairwise-IoU max + union max).

Contract: kernel(**inputs) takes FULL unsharded inputs, returns the FULL [2]
output. Internally shards the batch dim (8 images) across 8 NeuronCores, one
image per core; each core computes [max_prob_t, iou_max_of_its_image]; the
host gathers and averages the per-image iou scalars.

Per-core algorithm (N=4096 preds, M=2048 gts):
  Only preds with class==0 (~50/4096) matter, so the core first compacts
  person preds on-device: mask -> free-dim cumsum (tensor_tensor_scan) ->
  cross-partition exclusive prefix (triangular matmul) -> ranks -> one-hot
  -> accumulating PE matmuls gather person boxes into [128, 4].
  Slot layout (K=64): partition p holds person (p % 64) and gt-half (p // 64),
  so pairwise tiles are [128, 512] over 2 gt blocks.
  Gt/pred boxes are pre-split on the host into bf16 hi+lo parts (lossless to
  ~2^-18); two accumulating bf16 matmuls reconstruct fp32 in PSUM at the PE's
  1 cycle/row bf16 rate, dodging the 4 cycles/row fp32 tax.
  Pairwise: iw = min(px2,gx2)-max(px1,gx1) etc. via fused DVE ops; instead of
  iou = inter/uni per pair, rank by r = inter/(area_p+area_g+eps) which is a
  monotone transform of iou (iou = r/(1-r)); one reciprocal_approx_fast per
  block replaces per-pair division. Final: partition all-reduce max,
  iou = r*/(1-r*).
"""

import sys

sys.path.insert(0, "/opt/trn_rl_repo")

import contextlib

import numpy as np

import concourse.bacc as bacc
import concourse.mybir as mybir
from concourse import bass_isa
from concourse.tile import TileContext

F32 = mybir.dt.float32
BF16 = mybir.dt.bfloat16
I32 = mybir.dt.int32
Alu = mybir.AluOpType
Act = mybir.ActivationFunctionType

N = 4096  # preds per image
M = 2048  # gts per image
B = 8  # images == cores
U = 4096  # union entries
BIG = 1.0e30
EPS = 1.0e-9
FDB = 512  # gt-block free size for the pairwise phase (1 PSUM bank)
NCH = 32  # pred chunks of 128 (compaction contract dim)


def split_hi_lo(x: np.ndarray):
    """bf16 hi+lo decomposition of fp32 data, exact to ~2^-18 relative."""
    bf16 = mybir.dt.np(BF16)
    hi = x.astype(bf16)
    lo = (x - hi.astype(np.float32)).astype(bf16)
    return hi, lo


def build_kernel(K: int):
    """Build the per-core Bass module. K = person-slot count (64 or 128)."""
    assert K in (64, 128)
    nhalf = 128 // K  # gt-halves packed along partitions
    nblk = M // (nhalf * FDB)  # sequential gt blocks
    GW = 4 * M // nhalf  # gt row width (elems per half)

    nc = bacc.Bacc("TRN2", target_bir_lowering=False, debug=False)

    # packed inputs (host-side interleave): misc = [pred_classes | union_scores
    # | union_classes] as u32 rows; pb_hl/gt_hl = bf16 hi/lo interleaved per box
    misc = nc.dram_tensor("misc", [3, N], mybir.dt.uint32, kind="ExternalInput")
    pb_hl = nc.dram_tensor("pb_hl", [N, 8], BF16, kind="ExternalInput")
    gt_boxes = nc.dram_tensor("gt_boxes", [M, 4], F32, kind="ExternalInput")
    gt_hl = nc.dram_tensor("gt_hl", [M, 8], BF16, kind="ExternalInput")
    out = nc.dram_tensor("out", [2], F32, kind="ExternalOutput")

    # ---- constants baked into the NEFF ----
    # col 0-127: strict lower-tri (partition prefix); col 128-255: one-hot
    # rank targets (slot p matches rank (p % K) + 1) as bf16 pair-packed f32
    tri_np = (np.arange(128)[:, None] < np.arange(128)[None, :]).astype(np.float32)
    iota_np = np.tile(np.arange(1, K + 1, dtype=np.float32), 128 // K)
    iota_bf = np.broadcast_to(iota_np.astype(mybir.dt.np(BF16)), (128, 128))
    iota_as_f32 = np.ascontiguousarray(iota_bf).view(np.uint16).astype(np.uint32)
    merged = np.concatenate(
        [tri_np.view(np.uint32), (iota_as_f32[:, 0::2] | (iota_as_f32[:, 1::2] << 16))],
        axis=1,
    )  # [128, 192] u32: tri | bf16-packed iota
    sel_np = np.zeros((128, 128), dtype=np.float32)
    for p in range(128):
        sel_np[(p // K) % nhalf, p] = 1.0
    sel16_bits = np.ascontiguousarray(sel_np.astype(mybir.dt.np(BF16))).view(np.uint16).astype(np.uint32)
    sel16_packed = sel16_bits[:, 0::2] | (sel16_bits[:, 1::2] << 16)
    merged = np.concatenate(
        [merged, sel_np.view(np.uint32), sel16_packed], axis=1
    )  # [128, 384] u32: tri | iota | sel_f32 | sel_bf16
    const_merged = nc.inline_tensor(merged.astype(np.uint32), "consts")

    with TileContext(nc) as tc:
        ctx = contextlib.ExitStack()
        with ctx:
            const_pool = ctx.enter_context(tc.tile_pool(name="const", bufs=1))
            sb = ctx.enter_context(tc.tile_pool(name="sbuf", bufs=1))
            wrk = ctx.enter_context(tc.tile_pool(name="wrk", bufs=2))
            ohp = ctx.enter_context(tc.tile_pool(name="ohp", bufs=6))
            small = ctx.enter_context(tc.tile_pool(name="small", bufs=1))
            ps_g = ctx.enter_context(tc.tile_pool(name="ps_g", bufs=6, space="PSUM"))
            ps_s = ctx.enter_context(tc.tile_pool(name="ps_s", bufs=2, space="PSUM"))

            # ------- loads: few fat DMAs, rank-chain data first ------------
            misc_sb = sb.tile([128, 96], mybir.dt.uint32, tag="misc")
            nc.sync.dma_start(
                out=misc_sb[:],
                in_=misc.ap().rearrange("x (p f) -> p (x f)", p=128),
            )
            cls_sb = misc_sb[:, 0:32].bitcast(I32)
            uscore = misc_sb[:, 32:64].bitcast(F32)
            ucls = misc_sb[:, 64:96].bitcast(I32)
            cmerged = const_pool.tile([128, 384], mybir.dt.uint32, tag="cmerged")
            nc.scalar.dma_start(out=cmerged[:], in_=const_merged.ap())
            tri_sb = cmerged[:, 0:128].bitcast(F32)
            iota_sb = cmerged[:, 128:192].bitcast(BF16)
            sel_sb = cmerged[0:nhalf, 192:320].bitcast(F32)
            sel16_sb = cmerged[0:nhalf, 320:384].bitcast(BF16)
            ghl = sb.tile([nhalf, 2 * GW], BF16, tag="ghl")
            nc.sync.dma_start(
                out=ghl[:], in_=gt_hl.ap().flatten().rearrange("(q x) -> q x", q=nhalf)
            )
            grow = sb.tile([nhalf, GW], F32, tag="grow")
            nc.scalar.dma_start(
                out=grow[:],
                in_=gt_boxes.ap().flatten().rearrange("(q x) -> q x", q=nhalf),
            )
            pbhl = sb.tile([128, 256], BF16, tag="pbhl")
            nc.sync.dma_start(
                out=pbhl[:], in_=pb_hl.ap().flatten().rearrange("(p f) -> p f", p=128)
            )

            # ------- per-block gt areas on GpSimd (only needs grow) ---------
            ag_row = sb.tile([nhalf, M // nhalf], F32, tag="ag_row")
            for blk in range(nblk):
                g0 = 4 * FDB * blk
                a0 = FDB * blk
                wg_r = wrk.tile([nhalf, FDB], F32, tag="wg_r")
                nc.gpsimd.tensor_sub(
                    wg_r[:], grow[:, g0 + 2 : g0 + 4 * FDB : 4],
                    grow[:, g0 + 0 : g0 + 4 * FDB : 4],
                )
                hg_r = wrk.tile([nhalf, FDB], F32, tag="hg_r")
                nc.gpsimd.tensor_sub(
                    hg_r[:], grow[:, g0 + 3 : g0 + 4 * FDB : 4],
                    grow[:, g0 + 1 : g0 + 4 * FDB : 4],
                )
                nc.gpsimd.tensor_mul(ag_row[:, a0 : a0 + FDB], wg_r[:], hg_r[:])

            # ---------------- person mask + ranks ----------------
            m = small.tile([128, 32], F32, tag="m")
            nc.vector.tensor_scalar(m[:], cls_sb[:], 0, None, Alu.is_equal)
            s = small.tile([128, 32], F32, tag="s")
            nc.vector.tensor_tensor_scan(s[:], m[:], m[:], 0.0, Alu.add, Alu.max)
            pref_ps = ps_s.tile([128, 4], F32, tag="pss")
            nc.tensor.matmul(
                pref_ps[:, 0:1], tri_sb, s[:, 31:32], start=True, stop=True
            )
            q = small.tile([128, 32], F32, tag="q")
            nc.vector.scalar_tensor_tensor(
                q[:], s[:], pref_ps[:, 0:1], m[:], Alu.add, Alu.mult
            )

            # ------- gt coord broadcasts (PE, bf16 hi+lo accumulate) --------
            def bcast_coords(blk):
                g0 = blk * 8 * FDB
                tiles = []
                for c in (0, 2, 1, 3):  # x1, x2, y1, y2
                    gt_ps = ps_g.tile([128, FDB], F32, tag="g")
                    nc.tensor.matmul(
                        gt_ps[:], sel16_sb, ghl[:, g0 + c : g0 + 8 * FDB : 8],
                        start=True, stop=False,
                    )
                    nc.tensor.matmul(
                        gt_ps[:], sel16_sb, ghl[:, g0 + c + 4 : g0 + 8 * FDB : 8],
                        start=False, stop=True,
                    )
                    tiles.append(gt_ps)
                return tiles  # [x1, x2, y1, y2]

            def bcast_area(blk):
                ag_ps = ps_g.tile([128, FDB], F32, tag="g")
                nc.tensor.matmul(
                    ag_ps[:], sel_sb, ag_row[:, blk * FDB : (blk + 1) * FDB],
                    start=True, stop=True,
                )
                return ag_ps

            blk_tiles = {0: (bcast_coords(0), bcast_area(0))}

            # ---------------- compaction: one-hot + matmul gather -----------
            pc_ps = ps_s.tile([128, 4], F32, tag="pss")
            for f in range(NCH):
                oh = ohp.tile([128, 128], BF16, tag="oh")
                nc.vector.tensor_scalar(
                    oh[:], iota_sb, q[:, f : f + 1], None, Alu.is_equal
                )
                nc.tensor.matmul(
                    pc_ps[:], oh[:], pbhl[:, 8 * f : 8 * f + 4],
                    start=(f == 0), stop=False,
                )
                nc.tensor.matmul(
                    pc_ps[:], oh[:], pbhl[:, 8 * f + 4 : 8 * f + 8],
                    start=False, stop=(f == NCH - 1),
                )
            pc = small.tile([128, 4], F32, tag="pcs")
            nc.vector.tensor_copy(pc[:], pc_ps[:])
            px1, py1, px2, py2 = (pc[:, i : i + 1] for i in range(4))
            wp = small.tile([128, 1], F32, tag="wp")
            nc.vector.tensor_sub(wp[:], px2, px1)
            hp = small.tile([128, 1], F32, tag="hp")
            nc.vector.tensor_sub(hp[:], py2, py1)
            ap_eps = small.tile([128, 1], F32, tag="ap_eps")
            nc.vector.scalar_tensor_tensor(
                ap_eps[:], wp[:], EPS, hp[:], Alu.bypass, Alu.mult
            )
            nc.vector.tensor_scalar(ap_eps[:], ap_eps[:], EPS, None, Alu.add)

            # ---------------- per gt-block pairwise ----------------
            rmax_prev = None
            for blk in range(nblk):
                (gx1_ps, gx2_ps, gy1_ps, gy2_ps), ag_ps = blk_tiles[blk]
                # prefetch next block's broadcasts onto the PE queue now
                if blk + 1 < nblk:
                    blk_tiles[blk + 1] = (bcast_coords(blk + 1), bcast_area(blk + 1))

                m2x = wrk.tile([128, FDB], F32, tag="m2x")
                nc.vector.tensor_scalar(m2x[:], gx1_ps[:], px1, None, Alu.max)
                zx = wrk.tile([128, FDB], F32, tag="zx")
                nc.vector.scalar_tensor_tensor(
                    zx[:], gx2_ps[:], px2, m2x[:], Alu.min, Alu.subtract
                )
                m2y = wrk.tile([128, FDB], F32, tag="m2y")
                nc.vector.tensor_scalar(m2y[:], gy1_ps[:], py1, None, Alu.max)
                zy = wrk.tile([128, FDB], F32, tag="zy")
                nc.vector.scalar_tensor_tensor(
                    zy[:], gy2_ps[:], py2, m2y[:], Alu.min, Alu.subtract
                )
                ihr = wrk.tile([128, FDB], F32, tag="ihr")
                nc.scalar.activation(ihr[:], zy[:], Act.Relu)
                S_sb = wrk.tile([128, FDB], F32, tag="S")
                nc.scalar.activation(
                    S_sb[:], ag_ps[:], Act.Identity, bias=ap_eps[:], scale=1.0
                )
                srecip = wrk.tile([128, FDB], F32, tag="srecip")
                nc.vector.reciprocal_approx_fast(srecip[:], S_sb[:])
                inter = wrk.tile([128, FDB], F32, tag="inter")
                nc.vector.scalar_tensor_tensor(
                    inter[:], zx[:], 0.0, ihr[:], Alu.max, Alu.mult
                )
                r_buf = wrk.tile([128, FDB], F32, tag="r_buf")
                nc.vector.tensor_mul(r_buf[:], inter[:], srecip[:])
                rmax = small.tile([128, 1], F32, tag=f"rmax{blk}")
                nc.vector.tensor_reduce(rmax[:], r_buf[:], mybir.AxisListType.X, Alu.max)
                if rmax_prev is not None:
                    rmax2 = small.tile([128, 1], F32, tag=f"rmaxc{blk}")
                    nc.vector.tensor_max(rmax2[:], rmax[:], rmax_prev[:])
                    rmax = rmax2
                rmax_prev = rmax

            # ---------------- union max ----------------
            mu = small.tile([128, U // 128], I32, tag="mu")
            nc.vector.tensor_scalar(mu[:], ucls[:], 0, None, Alu.is_equal)
            um = small.tile([128, U // 128], F32, tag="um")
            nc.vector.memset(um[:], -BIG)
            nc.vector.copy_predicated(um[:], mu[:], uscore[:])
            fin = small.tile([128, 2], F32, tag="fin")
            nc.vector.tensor_reduce(fin[:, 0:1], um[:], mybir.AxisListType.X, Alu.max)

            # ---------------- final: iou = r/(1-r) per partition, one
            # fused cross-partition max over [umax | iou] ----------------
            one_m = small.tile([128, 1], F32, tag="one_m")
            nc.vector.tensor_scalar(
                one_m[:], rmax_prev[:], 1.0, -1.0, Alu.subtract, Alu.mult
            )
            rec = small.tile([128, 1], F32, tag="rec")
            nc.vector.reciprocal(rec[:], one_m[:])
            nc.vector.tensor_mul(fin[:, 1:2], rmax_prev[:], rec[:])
            fin_g = small.tile([128, 2], F32, tag="fin_g")
            nc.gpsimd.partition_all_reduce(
                fin_g[:], fin[:], 128, bass_isa.ReduceOp.max
            )
            nc.sync.dma_start(out=out.ap(), in_=fin_g[0:1, :])

    nc.compile()
    return nc


_KERNEL_CACHE = {}

# test/dev hooks
TRACE = False
LAST_RESULTS = None


def _get_kernel(K: int):
    if K not in _KERNEL_CACHE:
        _KERNEL_CACHE[K] = build_kernel(K)
    return _KERNEL_CACHE[K]


def make_in_maps(pred_boxes, pred_classes, gt_boxes, union_scores, union_classes):
    misc_shared = np.stack(
        [
            np.zeros(U, np.uint32),  # per-image, filled below
            union_scores.view(np.uint32),
            union_classes.view(np.uint32),
        ]
    )
    in_maps = []
    for b in range(B):
        ghi, glo = split_hi_lo(gt_boxes[b])
        phi, plo = split_hi_lo(pred_boxes[b])
        misc = misc_shared.copy()
        misc[0] = pred_classes[b].view(np.uint32)
        in_maps.append(
            {
                "misc": misc,
                "pb_hl": np.concatenate([phi, plo], axis=1),
                "gt_boxes": gt_boxes[b],
                "gt_hl": np.concatenate([ghi, glo], axis=1),
            }
        )
    return in_maps


def kernel(pred_boxes, pred_scores, pred_classes, gt_boxes, union_scores, union_classes):
    from concourse.bass_utils import run_bass_kernel_spmd

    pred_boxes = np.ascontiguousarray(np.asarray(pred_boxes, dtype=np.float32))
    pred_classes = np.ascontiguousarray(np.asarray(pred_classes, dtype=np.int32))
    gt_boxes = np.ascontiguousarray(np.asarray(gt_boxes, dtype=np.float32))
    union_scores = np.ascontiguousarray(np.asarray(union_scores, dtype=np.float32))
    union_classes = np.ascontiguousarray(np.asarray(union_classes, dtype=np.int32))

    max_persons = int((pred_classes == 0).sum(axis=1).max())
    K = 64 if max_persons <= 64 else 128
    nc = _get_kernel(K)

    in_maps = make_in_maps(pred_boxes, pred_classes, gt_boxes, union_scores, union_classes)
    res = run_bass_kernel_spmd(nc, in_maps, list(range(B)), trace=TRACE)
    global LAST_RESULTS
    LAST_RESULTS = res
    outs = np.stack([res.results[b]["out"] for b in range(B)])  # [B, 2]
    max_prob = outs[0, 0]
    max_iou = outs[:, 1].mean(dtype=np.float32)
    return np.array([max_prob, max_iou], dtype=np.float32)


# revision 25
# speedup vs baseline: 1.6958x; 1.2525x over previous
"""Trainium2 Bass kernel for nn_MaxExtractor (masked pairwise-IoU max + union max).

Contract: kernel(**inputs) takes FULL unsharded inputs, returns the FULL [2]
output. Internally shards the batch dim (8 images) across 8 NeuronCores, one
image per core; each core computes [max_prob_t, iou_max_of_its_image]; the
host gathers and averages the per-image iou scalars.

Per-core algorithm (N=4096 preds, M=2048 gts):
  Only preds with class==0 (~50/4096) matter, so the core first compacts
  person preds on-device: mask -> free-dim cumsum (tensor_tensor_scan) ->
  cross-partition exclusive prefix (triangular matmul) -> ranks -> one-hot
  -> accumulating PE matmuls gather person boxes into [128, 4].
  Slot layout (K=64): partition p holds person (p % 64) and gt-half (p // 64),
  so pairwise tiles are [128, 512] over 2 gt blocks.
  Gt/pred boxes are pre-split on the host into bf16 hi+lo parts (lossless to
  ~2^-18); two accumulating bf16 matmuls reconstruct fp32 in PSUM at the PE's
  1 cycle/row bf16 rate, dodging the 4 cycles/row fp32 tax.
  Pairwise: iw = min(px2,gx2)-max(px1,gx1) etc. via fused DVE ops; instead of
  iou = inter/uni per pair, rank by r = inter/(area_p+area_g+eps) which is a
  monotone transform of iou (iou = r/(1-r)); one reciprocal_approx_fast per
  block replaces per-pair division. Final: partition all-reduce max,
  iou = r*/(1-r*).
"""

import sys

sys.path.insert(0, "/opt/trn_rl_repo")

import contextlib

import numpy as np

import concourse.bacc as bacc
import concourse.mybir as mybir
from concourse import bass_isa
from concourse.tile import TileContext

F32 = mybir.dt.float32
BF16 = mybir.dt.bfloat16
I32 = mybir.dt.int32
Alu = mybir.AluOpType
Act = mybir.ActivationFunctionType

N = 4096  # preds per image
M = 2048  # gts per image
B = 8  # images == cores
U = 4096  # union entries
BIG = 1.0e30
EPS = 1.0e-9
FDB = 512  # gt-block free size for the pairwise phase (1 PSUM bank)
NCH = 32  # pred chunks of 128 (compaction contract dim)


def split_hi_lo(x: np.ndarray):
    """bf16 hi+lo decomposition of fp32 data, exact to ~2^-18 relative."""
    bf16 = mybir.dt.np(BF16)
    hi = x.astype(bf16)
    lo = (x - hi.astype(np.float32)).astype(bf16)
    return hi, lo


def build_kernel(K: int):
    """Build the per-core Bass module. K = person-slot count (64 or 128)."""
    assert K in (64, 128)
    nhalf = 128 // K  # gt-halves packed along partitions
    nblk = M // (nhalf * FDB)  # sequential gt blocks
    GW = 4 * M // nhalf  # gt row width (elems per half)

    nc = bacc.Bacc("TRN2", target_bir_lowering=False, debug=False)

    # packed inputs (host-side interleave): misc = [pred_classes | union_scores
    # | union_classes] as u32 rows; pb_hl/gt_hl = bf16 hi/lo interleaved per box
    misc = nc.dram_tensor("misc", [3, N], mybir.dt.uint32, kind="ExternalInput")
    pb_hl = nc.dram_tensor("pb_hl", [N, 8], BF16, kind="ExternalInput")
    gt_boxes = nc.dram_tensor("gt_boxes", [M, 4], F32, kind="ExternalInput")
    gt_hl = nc.dram_tensor("gt_hl", [M, 8], BF16, kind="ExternalInput")
    out = nc.dram_tensor("out", [2], F32, kind="ExternalOutput")

    # ---- constants baked into the NEFF ----
    # col 0-127: strict lower-tri (partition prefix); col 128-255: one-hot
    # rank targets (slot p matches rank (p % K) + 1) as bf16 pair-packed f32
    tri_np = (np.arange(128)[:, None] < np.arange(128)[None, :]).astype(np.float32)
    iota_np = np.tile(np.arange(1, K + 1, dtype=np.float32), 128 // K)
    iota_bf = np.broadcast_to(iota_np.astype(mybir.dt.np(BF16)), (128, 128))
    iota_as_f32 = np.ascontiguousarray(iota_bf).view(np.uint16).astype(np.uint32)
    merged = np.concatenate(
        [tri_np.view(np.uint32), (iota_as_f32[:, 0::2] | (iota_as_f32[:, 1::2] << 16))],
        axis=1,
    )  # [128, 192] u32: tri | bf16-packed iota
    sel_np = np.zeros((128, 128), dtype=np.float32)
    for p in range(128):
        sel_np[(p // K) % nhalf, p] = 1.0
    sel16_bits = np.ascontiguousarray(sel_np.astype(mybir.dt.np(BF16))).view(np.uint16).astype(np.uint32)
    sel16_packed = sel16_bits[:, 0::2] | (sel16_bits[:, 1::2] << 16)
    merged = np.concatenate(
        [merged, sel_np.view(np.uint32), sel16_packed], axis=1
    )  # [128, 384] u32: tri | iota | sel_f32 | sel_bf16
    const_merged = nc.inline_tensor(merged.astype(np.uint32), "consts")

    with TileContext(nc) as tc:
        ctx = contextlib.ExitStack()
        with ctx:
            const_pool = ctx.enter_context(tc.tile_pool(name="const", bufs=1))
            sb = ctx.enter_context(tc.tile_pool(name="sbuf", bufs=1))
            wrk = ctx.enter_context(tc.tile_pool(name="wrk", bufs=2))
            ohp = ctx.enter_context(tc.tile_pool(name="ohp", bufs=32))
            small = ctx.enter_context(tc.tile_pool(name="small", bufs=1))
            ps_g = ctx.enter_context(tc.tile_pool(name="ps_g", bufs=6, space="PSUM"))
            ps_s = ctx.enter_context(tc.tile_pool(name="ps_s", bufs=2, space="PSUM"))

            # ------- loads: few fat DMAs, rank-chain data first ------------
            misc_sb = sb.tile([128, 96], mybir.dt.uint32, tag="misc")
            nc.sync.dma_start(
                out=misc_sb[:],
                in_=misc.ap().rearrange("x (p f) -> p x f", p=128),
            )
            cls_sb = misc_sb[:, 0:32].bitcast(I32)
            uscore = misc_sb[:, 32:64].bitcast(F32)
            ucls = misc_sb[:, 64:96].bitcast(I32)
            cmerged = const_pool.tile([128, 384], mybir.dt.uint32, tag="cmerged")
            nc.scalar.dma_start(out=cmerged[:], in_=const_merged.ap())
            tri_sb = cmerged[:, 0:128].bitcast(F32)
            iota_sb = cmerged[:, 128:192].bitcast(BF16)
            sel_sb = cmerged[0:nhalf, 192:320].bitcast(F32)
            sel16_sb = cmerged[0:nhalf, 320:384].bitcast(BF16)
            ghl = sb.tile([nhalf, 2 * GW], BF16, tag="ghl")
            nc.sync.dma_start(
                out=ghl[:], in_=gt_hl.ap().flatten().rearrange("(q x) -> q x", q=nhalf)
            )
            grow = sb.tile([nhalf, GW], F32, tag="grow")
            nc.scalar.dma_start(
                out=grow[:],
                in_=gt_boxes.ap().flatten().rearrange("(q x) -> q x", q=nhalf),
            )
            pbhl = sb.tile([128, 256], BF16, tag="pbhl")
            nc.sync.dma_start(
                out=pbhl[:], in_=pb_hl.ap().flatten().rearrange("(p f) -> p f", p=128)
            )

            # ------- per-block gt areas on GpSimd (only needs grow) ---------
            ag_row = sb.tile([nhalf, M // nhalf], F32, tag="ag_row")
            for blk in range(nblk):
                g0 = 4 * FDB * blk
                a0 = FDB * blk
                wg_r = wrk.tile([nhalf, FDB], F32, tag="wg_r")
                nc.gpsimd.tensor_sub(
                    wg_r[:], grow[:, g0 + 2 : g0 + 4 * FDB : 4],
                    grow[:, g0 + 0 : g0 + 4 * FDB : 4],
                )
                hg_r = wrk.tile([nhalf, FDB], F32, tag="hg_r")
                nc.gpsimd.tensor_sub(
                    hg_r[:], grow[:, g0 + 3 : g0 + 4 * FDB : 4],
                    grow[:, g0 + 1 : g0 + 4 * FDB : 4],
                )
                nc.gpsimd.tensor_mul(ag_row[:, a0 : a0 + FDB], wg_r[:], hg_r[:])

            # ---------------- person mask + ranks ----------------
            m = small.tile([128, 32], F32, tag="m")
            nc.vector.tensor_scalar(m[:], cls_sb[:], 0, None, Alu.is_equal)
            s = small.tile([128, 32], F32, tag="s")
            nc.vector.tensor_tensor_scan(s[:], m[:], m[:], 0.0, Alu.add, Alu.max)
            pref_ps = ps_s.tile([128, 4], F32, tag="pss")
            nc.tensor.matmul(
                pref_ps[:, 0:1], tri_sb, s[:, 31:32], start=True, stop=True
            )
            q = small.tile([128, 32], F32, tag="q")
            nc.vector.scalar_tensor_tensor(
                q[:], s[:], pref_ps[:, 0:1], m[:], Alu.add, Alu.mult
            )

            # ---------------- compaction: one-hot + matmul gather -----------
            pc_ps = ps_s.tile([128, 4], F32, tag="pss")
            for f in range(NCH):
                oh = ohp.tile([128, 128], BF16, tag="oh")
                nc.vector.tensor_scalar(
                    oh[:], iota_sb, q[:, f : f + 1], None, Alu.is_equal
                )
                nc.tensor.matmul(
                    pc_ps[:], oh[:], pbhl[:, 8 * f : 8 * f + 4],
                    start=(f == 0), stop=False,
                )
                nc.tensor.matmul(
                    pc_ps[:], oh[:], pbhl[:, 8 * f + 4 : 8 * f + 8],
                    start=False, stop=(f == NCH - 1),
                )
            pc = small.tile([128, 4], F32, tag="pcs")
            nc.vector.tensor_copy(pc[:], pc_ps[:])
            px1, py1, px2, py2 = (pc[:, i : i + 1] for i in range(4))
            wp = small.tile([128, 1], F32, tag="wp")
            nc.vector.tensor_sub(wp[:], px2, px1)
            hp = small.tile([128, 1], F32, tag="hp")
            nc.vector.tensor_sub(hp[:], py2, py1)
            ap_eps = small.tile([128, 1], F32, tag="ap_eps")
            nc.vector.scalar_tensor_tensor(
                ap_eps[:], wp[:], EPS, hp[:], Alu.bypass, Alu.mult
            )
            nc.vector.tensor_scalar(ap_eps[:], ap_eps[:], EPS, None, Alu.add)

            # ------- gt coord broadcasts (PE, bf16 hi+lo accumulate) --------
            def bcast_coords(blk):
                g0 = blk * 8 * FDB
                tiles = []
                for c in (0, 2, 1, 3):  # x1, x2, y1, y2
                    gt_ps = ps_g.tile([128, FDB], F32, tag="g")
                    nc.tensor.matmul(
                        gt_ps[:], sel16_sb, ghl[:, g0 + c : g0 + 8 * FDB : 8],
                        start=True, stop=False,
                    )
                    nc.tensor.matmul(
                        gt_ps[:], sel16_sb, ghl[:, g0 + c + 4 : g0 + 8 * FDB : 8],
                        start=False, stop=True,
                    )
                    tiles.append(gt_ps)
                return tiles  # [x1, x2, y1, y2]

            def bcast_area(blk):
                ag_ps = ps_g.tile([128, FDB], F32, tag="g")
                nc.tensor.matmul(
                    ag_ps[:], sel_sb, ag_row[:, blk * FDB : (blk + 1) * FDB],
                    start=True, stop=True,
                )
                return ag_ps

            blk_tiles = {0: (bcast_coords(0), bcast_area(0))}

            # ---------------- union max ----------------
            mu = small.tile([128, U // 128], I32, tag="mu")
            nc.vector.tensor_scalar(mu[:], ucls[:], 0, None, Alu.is_equal)
            um = small.tile([128, U // 128], F32, tag="um")
            nc.vector.memset(um[:], -BIG)
            nc.vector.copy_predicated(um[:], mu[:], uscore[:])
            # ---------------- per gt-block pairwise ----------------
            r_all = sb.tile([128, nblk * FDB], F32, tag="r_all")
            for blk in range(nblk):
                (gx1_ps, gx2_ps, gy1_ps, gy2_ps), ag_ps = blk_tiles[blk]
                # prefetch next block's broadcasts onto the PE queue now
                if blk + 1 < nblk:
                    blk_tiles[blk + 1] = (bcast_coords(blk + 1), bcast_area(blk + 1))

                m2x = wrk.tile([128, FDB], F32, tag="m2x")
                nc.vector.tensor_scalar(m2x[:], gx1_ps[:], px1, None, Alu.max)
                zx = wrk.tile([128, FDB], F32, tag="zx")
                nc.vector.scalar_tensor_tensor(
                    zx[:], gx2_ps[:], px2, m2x[:], Alu.min, Alu.subtract
                )
                m2y = wrk.tile([128, FDB], F32, tag="m2y")
                nc.vector.tensor_scalar(m2y[:], gy1_ps[:], py1, None, Alu.max)
                zy = wrk.tile([128, FDB], F32, tag="zy")
                nc.vector.scalar_tensor_tensor(
                    zy[:], gy2_ps[:], py2, m2y[:], Alu.min, Alu.subtract
                )
                ihr = wrk.tile([128, FDB], F32, tag="ihr")
                nc.scalar.activation(ihr[:], zy[:], Act.Relu)
                S_sb = wrk.tile([128, FDB], F32, tag="S")
                nc.scalar.activation(
                    S_sb[:], ag_ps[:], Act.Identity, bias=ap_eps[:], scale=1.0
                )
                srecip = wrk.tile([128, FDB], F32, tag="srecip")
                nc.vector.reciprocal_approx_fast(srecip[:], S_sb[:])
                inter = wrk.tile([128, FDB], F32, tag="inter")
                nc.vector.scalar_tensor_tensor(
                    inter[:], zx[:], 0.0, ihr[:], Alu.max, Alu.mult
                )
                nc.vector.tensor_mul(
                    r_all[:, blk * FDB : (blk + 1) * FDB], inter[:], srecip[:]
                )

            fin = small.tile([128, 2], F32, tag="fin")
            nc.vector.tensor_reduce(fin[:, 0:1], um[:], mybir.AxisListType.X, Alu.max)
            rmax_prev = small.tile([128, 1], F32, tag="rmaxall")
            nc.vector.tensor_reduce(
                rmax_prev[:], r_all[:], mybir.AxisListType.X, Alu.max
            )

            # ---------------- final: iou = r/(1-r) per partition, one
            # fused cross-partition max over [umax | iou] ----------------
            one_m = small.tile([128, 1], F32, tag="one_m")
            nc.vector.tensor_scalar(
                one_m[:], rmax_prev[:], 1.0, -1.0, Alu.subtract, Alu.mult
            )
            rec = small.tile([128, 1], F32, tag="rec")
            nc.vector.reciprocal(rec[:], one_m[:])
            nc.vector.tensor_mul(fin[:, 1:2], rmax_prev[:], rec[:])
            fin_g = small.tile([128, 2], F32, tag="fin_g")
            nc.gpsimd.partition_all_reduce(
                fin_g[:], fin[:], 128, bass_isa.ReduceOp.max
            )
            nc.sync.dma_start(out=out.ap(), in_=fin_g[0:1, :])

    nc.compile()
    return nc


_KERNEL_CACHE = {}

# test/dev hooks
TRACE = False
LAST_RESULTS = None


def _get_kernel(K: int):
    if K not in _KERNEL_CACHE:
        _KERNEL_CACHE[K] = build_kernel(K)
    return _KERNEL_CACHE[K]


def make_in_maps(pred_boxes, pred_classes, gt_boxes, union_scores, union_classes):
    misc_shared = np.stack(
        [
            np.zeros(U, np.uint32),  # per-image, filled below
            union_scores.view(np.uint32),
            union_classes.view(np.uint32),
        ]
    )
    in_maps = []
    for b in range(B):
        ghi, glo = split_hi_lo(gt_boxes[b])
        phi, plo = split_hi_lo(pred_boxes[b])
        misc = misc_shared.copy()
        misc[0] = pred_classes[b].view(np.uint32)
        in_maps.append(
            {
                "misc": misc,
                "pb_hl": np.concatenate([phi, plo], axis=1),
                "gt_boxes": gt_boxes[b],
                "gt_hl": np.concatenate([ghi, glo], axis=1),
            }
        )
    return in_maps


def kernel(pred_boxes, pred_scores, pred_classes, gt_boxes, union_scores, union_classes):
    from concourse.bass_utils import run_bass_kernel_spmd

    pred_boxes = np.ascontiguousarray(np.asarray(pred_boxes, dtype=np.float32))
    pred_classes = np.ascontiguousarray(np.asarray(pred_classes, dtype=np.int32))
    gt_boxes = np.ascontiguousarray(np.asarray(gt_boxes, dtype=np.float32))
    union_scores = np.ascontiguousarray(np.asarray(union_scores, dtype=np.float32))
    union_classes = np.ascontiguousarray(np.asarray(union_classes, dtype=np.int32))

    max_persons = int((pred_classes == 0).sum(axis=1).max())
    K = 64 if max_persons <= 64 else 128
    nc = _get_kernel(K)

    in_maps = make_in_maps(pred_boxes, pred_classes, gt_boxes, union_scores, union_classes)
    res = run_bass_kernel_spmd(nc, in_maps, list(range(B)), trace=TRACE)
    global LAST_RESULTS
    LAST_RESULTS = res
    outs = np.stack([res.results[b]["out"] for b in range(B)])  # [B, 2]
    max_prob = outs[0, 0]
    max_iou = outs[:, 1].mean(dtype=np.float32)
    return np.array([max_prob, max_iou], dtype=np.float32)
